# revision 17
# baseline (speedup 1.0000x reference)
"""Trainium2 Bass kernel for nn_CorrectSplineLinear (embedding_lookup regime).

Math: reference computes
    W[o,t,f] = sum_c interp[o,t,c] * E[c,f]        (interp = piecewise-linear in t)
    out[o,b,t] = sum_f x[b,f] * W[o,t,f]
which collapses algebraically to
    y[c,b]    = sum_f E[c,f] * x[b,f]              ([128,128] matmul)
    Z[o,s,b]  = sum_c cv[o,s,c] * y[c,b]           ([128,128] matmul per core)
    out[o,b,t]= Z[o,j(t),b] + tl(t)*(Z[o,j(t)+1,b] - Z[o,j(t),b])
so no [O,I,I] weight is ever materialized.  The kernel is memory-bound on
writing the [256,128,512] fp32 output (8 MiB per core across 8 cores); the
total time is essentially (time until the first output row is ready) +
(8 MiB at HBM write rate) + fixed tail, so the front of the pipeline is
aggressively shortened:
  * inputs arrive as small chunked DMAs on two HWDGE rings so the y matmul
    starts as soon as the first 128KB lands
  * dZ is folded into the Z matmul: GpSimd computes dcvT = cvT[:,i+1]-cvT[:,i]
    once, and the PE produces [Z | dZ] in one pass (split 16/112 columns so
    output row 0 unblocks early), leaving one ScalarE copy on the chain
  * the expansion (out = tl*dZ + Z, two per-partition scalars per
    instruction) is split per spline segment across VectorE, ScalarE and
    GpSimdE, and the first 8 output rows are stored row-at-a-time so the
    HBM write stream starts as early as possible

Sharding: out_features O=256 split across 8 cores (32 rows each); x and E
replicated; each core gets its control_values slice pre-transposed.
"""

import sys
from contextlib import ExitStack

import numpy as np

try:
    import concourse.bass as bass
except ImportError:  # fresh grading dir: concourse lives in the repo checkout
    sys.path.insert(0, "/opt/trn_rl_repo")
    import concourse.bass as bass

import concourse.bacc as bacc
import concourse.mybir as mybir
import concourse.tile as tile
from concourse.bass_utils import run_bass_kernel_spmd

N_CORES = 8
O, I, K, C, B = 256, 512, 3, 128, 128
OL = O // N_CORES  # 32 output rows per core
NS = K + 1  # 4 control values per output row
NZ = OL * NS  # 128 Z columns per core
F32 = mybir.dt.float32

# ---- spline geometry (input-independent, mirrors reference arithmetic) ----
_t = np.linspace(0.0, 1.0, I).astype(np.float32)
_ts = (_t * np.float32(K)).astype(np.float32)
_j = np.clip(np.floor(_ts), 0.0, float(K - 1)).astype(np.int32)
_TL = (_ts - _j.astype(np.float32)).astype(np.float32)  # [I] local coord in segment
_b0 = int(np.searchsorted(_j, 1))  # first t index in segment 1
_b1 = int(np.searchsorted(_j, 2))  # first t index in segment 2
# Disjoint per-segment spans; each output row's three segment ops run on
# three different engines in parallel (ScalarE / VectorE / GpSimdE).
_SPANS = [(0, 0, _b0), (1, _b0, _b1), (2, _b1, I)]  # (segment j, t0, t1)
_SPAN_ENG = ["a", "v", "g"]

# ---- packed-input column layout ([128, _TOT] fp32) ----
# 4 chunk-pairs [xT_k | eT_k], then cvT, a device-computed dcvT scratch
# region, then tl broadcast.
_CV0 = 4 * (B + C)  # 1024
_DCV0 = _CV0 + NZ  # 1152 (not DMA'd; GpSimd writes cvT[:,i+1]-cvT[:,i])
_TL0 = _DCV0 + NZ  # 1280
_TOT = _TL0 + I  # 1792

GROUP = 4  # output rows per store DMA (4*128*512*4B = 1 MiB)
NGRP = OL // GROUP
EARLY_GROUPS = 2  # first groups store per-row (256KB) so the write stream starts ASAP
ZSPLIT = NS * GROUP  # Z columns needed by the first store group

_cache: dict = {}


def _build_nc():
    nc = bacc.Bacc("TRN2", target_bir_lowering=False, debug=False, num_devices=N_CORES)
    pk_d = nc.dram_tensor("pk", [128, _TOT], F32, kind="ExternalInput")
    out_d = nc.dram_tensor("out", [OL, B, I], F32, kind="ExternalOutput")

    with tile.TileContext(nc) as tc, ExitStack() as ctx:
        constp = ctx.enter_context(tc.tile_pool(name="const", bufs=1))
        psump = ctx.enter_context(
            tc.tile_pool(name="psum", bufs=1, space=bass.MemorySpace.PSUM)
        )
        outp = ctx.enter_context(tc.tile_pool(name="outs", bufs=1))

        pk = constp.tile([128, _TOT], F32)
        # input loads split across both HWDGE rings (SyncE + ScalarE) so the
        # issue overhead overlaps and the first matmul starts earliest
        nc.sync.dma_start(pk[:, 0:256], pk_d[:, 0:256])
        nc.scalar.dma_start(pk[:, 256:512], pk_d[:, 256:512])
        nc.sync.dma_start(pk[:, 512:768], pk_d[:, 512:768])
        nc.scalar.dma_start(pk[:, 768:1024], pk_d[:, 768:1024])
        nc.scalar.dma_start(pk[:, _CV0 : _CV0 + NZ], pk_d[:, _CV0 : _CV0 + NZ])
        nc.sync.dma_start(pk[:, _TL0:_TOT], pk_d[:, _TL0:_TOT])

        # dcvT[c, i] = cvT[c, i+1] - cvT[c, i]  (GpSimd, off the critical path)
        nc.gpsimd.memset(pk[:, _DCV0 + NZ - 1 : _DCV0 + NZ], 0.0)  # last dcv col
        nc.gpsimd.tensor_sub(
            pk[:, _DCV0 : _DCV0 + NZ - 1],
            pk[:, _CV0 + 1 : _CV0 + NZ],
            pk[:, _CV0 : _CV0 + NZ - 1],
        )

        # y[c,b] = sum_f E[c,f] x[b,f]: accumulate over 4 chunks of f.
        y_ps = psump.tile([128, B], F32)
        for k in range(4):
            base = k * 256
            nc.tensor.matmul(
                y_ps[:],
                pk[:, base + B : base + B + C],  # lhsT [f_chunk, c]
                pk[:, base : base + B],  # rhs  [f_chunk, b]
                start=(k == 0),
                stop=(k == 3),
            )
        y_sb = constp.tile([128, B], F32)
        nc.vector.tensor_copy(y_sb[:], y_ps[:])

        # One PE pass produces both ZT[b, o*4+s] and dZT[b, o*4+s] by using
        # rhs = [cvT block | dcvT block] (2-block access pattern).  Split
        # 16/112 columns so output row 0 unblocks early.
        cvd = pk[:, _CV0 : _CV0 + 2 * NZ].rearrange("p (u c) -> p u c", u=2)
        ztdz = constp.tile([128, 2 * NZ], F32)  # [ZT | dZT]
        ztdz_v = ztdz[:].rearrange("p (u c) -> p u c", u=2)
        zz_ps1 = psump.tile([128, 2 * ZSPLIT], F32)
        zz_ps2 = psump.tile([128, 2 * (NZ - ZSPLIT)], F32)

        nc.tensor.matmul(
            zz_ps1[:], y_sb[:], cvd[:, :, 0:ZSPLIT], start=True, stop=True
        )
        nc.vector.tensor_copy(
            ztdz_v[:, :, 0:ZSPLIT], zz_ps1[:].rearrange("p (u c) -> p u c", u=2)
        )

        def _ztdz_rest():
            nc.tensor.matmul(
                zz_ps2[:], y_sb[:], cvd[:, :, ZSPLIT:NZ], start=True, stop=True
            )
            nc.scalar.activation(
                ztdz_v[:, :, ZSPLIT:NZ],
                zz_ps2[:].rearrange("p (u c) -> p u c", u=2),
                mybir.ActivationFunctionType.Identity,
            )

        outs = outp.tile([128, OL * I], F32)
        tl_ap = pk[:, _TL0 : _TL0 + I]

        for g in range(NGRP):
            if g == 1:
                _ztdz_rest()
            for oi in range(GROUP):
                o = g * GROUP + oi
                col = o * I
                zc = NS * o
                for (j, t0, t1), eng in zip(_SPANS, _SPAN_ENG):
                    if eng == "a":
                        nc.scalar.activation(
                            outs[:, col + t0 : col + t1],
                            tl_ap[:, t0:t1],
                            mybir.ActivationFunctionType.Identity,
                            bias=ztdz[:, zc + j : zc + j + 1],
                            scale=ztdz[:, NZ + zc + j : NZ + zc + j + 1],
                        )
                    else:
                        veng = nc.vector if eng == "v" else nc.gpsimd
                        veng.tensor_scalar(
                            outs[:, col + t0 : col + t1],
                            tl_ap[:, t0:t1],
                            ztdz[:, NZ + zc + j : NZ + zc + j + 1],
                            ztdz[:, zc + j : zc + j + 1],
                            mybir.AluOpType.mult,
                            mybir.AluOpType.add,
                        )
                if g < EARLY_GROUPS:
                    nc.sync.dma_start(
                        out_d[o : o + 1].rearrange("o b t -> b o t"),
                        outs[:, o * I : (o + 1) * I].rearrange("p (o t) -> p o t", o=1),
                    )
            if g >= EARLY_GROUPS:
                nc.sync.dma_start(
                    out_d[g * GROUP : (g + 1) * GROUP].rearrange("o b t -> b o t"),
                    outs[:, g * GROUP * I : (g + 1) * GROUP * I].rearrange(
                        "p (o t) -> p o t", o=GROUP
                    ),
                )

    nc.compile()
    return nc


def _get_nc():
    if "nc" not in _cache:
        _cache["nc"] = _build_nc()
    return _cache["nc"]


def _pack_inputs(x, control_values, expansion_matrix):
    x = np.ascontiguousarray(x, dtype=np.float32)
    cv = np.ascontiguousarray(control_values, dtype=np.float32)
    E = np.ascontiguousarray(expansion_matrix, dtype=np.float32)

    base = np.zeros((128, _TOT), dtype=np.float32)
    for k in range(4):
        base[:, k * 256 : k * 256 + B] = x[:, k * 128 : (k + 1) * 128].T
        base[:, k * 256 + B : k * 256 + B + C] = E[:, k * 128 : (k + 1) * 128].T
    base[:, _TL0 : _TL0 + I] = _TL[None, :]

    in_maps = []
    for core in range(N_CORES):
        m = base.copy()
        slab = cv[core * OL : (core + 1) * OL].reshape(OL * NS, C)  # [(o,s), c]
        m[:, _CV0 : _CV0 + NZ] = slab.T
        in_maps.append({"pk": m})
    return in_maps


def _run(in_maps, trace=False):
    nc = _get_nc()
    return run_bass_kernel_spmd(
        nc, in_maps, core_ids=list(range(N_CORES)), trace=trace
    )


def kernel(x, control_points, control_values, expansion_matrix):
    in_maps = _pack_inputs(x, control_values, expansion_matrix)
    res = _run(in_maps, trace=False)
    return np.concatenate([r["out"] for r in res.results], axis=0)


def kernel_traced(x, control_points, control_values, expansion_matrix):
    """Same as kernel() but profiles on HW; returns (out, BassKernelResults)."""
    in_maps = _pack_inputs(x, control_values, expansion_matrix)
    res = _run(in_maps, trace=True)
    out = np.concatenate([r["out"] for r in res.results], axis=0)
    return out, res


# revision 18
# speedup vs baseline: 1.0743x; 1.0743x over previous
"""Trainium2 Bass kernel for nn_CorrectSplineLinear (embedding_lookup regime).

Math: reference computes
    W[o,t,f] = sum_c interp[o,t,c] * E[c,f]        (interp = piecewise-linear in t)
    out[o,b,t] = sum_f x[b,f] * W[o,t,f]
which collapses algebraically to
    y[c,b]    = sum_f E[c,f] * x[b,f]              ([128,128] matmul)
    Z[o,s,b]  = sum_c cv[o,s,c] * y[c,b]           ([128,128] matmul per core)
    out[o,b,t]= Z[o,j(t),b] + tl(t)*(Z[o,j(t)+1,b] - Z[o,j(t),b])
so no [O,I,I] weight is ever materialized.  The kernel is memory-bound on
writing the [256,128,512] fp32 output (8 MiB per core across 8 cores); the
total time is essentially (time until the first output row is ready) +
(8 MiB at HBM write rate) + fixed tail, so the front of the pipeline is
aggressively shortened:
  * inputs arrive as small chunked DMAs on two HWDGE rings so the y matmul
    starts as soon as the first 128KB lands
  * dZ is folded into the Z matmul: GpSimd computes dcvT = cvT[:,i+1]-cvT[:,i]
    once, and the PE produces [Z | dZ] in one pass (split 16/112 columns so
    output row 0 unblocks early), leaving one ScalarE copy on the chain
  * the expansion (out = tl*dZ + Z, two per-partition scalars per
    instruction) is split per spline segment across VectorE, ScalarE and
    GpSimdE, and the first 8 output rows are stored row-at-a-time so the
    HBM write stream starts as early as possible

Sharding: out_features O=256 split across 8 cores (32 rows each); x and E
replicated; each core gets its control_values slice pre-transposed.
"""

import sys
from contextlib import ExitStack

import numpy as np

try:
    import concourse.bass as bass
except ImportError:  # fresh grading dir: concourse lives in the repo checkout
    sys.path.insert(0, "/opt/trn_rl_repo")
    import concourse.bass as bass

import concourse.bacc as bacc
import concourse.mybir as mybir
import concourse.tile as tile
from concourse.bass_utils import run_bass_kernel_spmd

N_CORES = 8
O, I, K, C, B = 256, 512, 3, 128, 128
OL = O // N_CORES  # 32 output rows per core
NS = K + 1  # 4 control values per output row
NZ = OL * NS  # 128 Z columns per core
F32 = mybir.dt.float32

# ---- spline geometry (input-independent, mirrors reference arithmetic) ----
_t = np.linspace(0.0, 1.0, I).astype(np.float32)
_ts = (_t * np.float32(K)).astype(np.float32)
_j = np.clip(np.floor(_ts), 0.0, float(K - 1)).astype(np.int32)
_TL = (_ts - _j.astype(np.float32)).astype(np.float32)  # [I] local coord in segment
_b0 = int(np.searchsorted(_j, 1))  # first t index in segment 1
_b1 = int(np.searchsorted(_j, 2))  # first t index in segment 2
# Disjoint per-segment spans; each output row's three segment ops run on
# three different engines in parallel (ScalarE / VectorE / GpSimdE).
_SPANS = [(0, 0, _b0), (1, _b0, _b1), (2, _b1, I)]  # (segment j, t0, t1)
_SPAN_ENG = ["a", "v", "g"]

# ---- packed-input column layout ([128, _TOT] fp32) ----
# 4 chunk-pairs [xT_k | eT_k], then cvT, a device-computed dcvT scratch
# region, then tl broadcast.
_CV0 = 4 * (B + C)  # 1024
_DCV0 = _CV0 + NZ  # 1152 (not DMA'd; GpSimd writes cvT[:,i+1]-cvT[:,i])
_TL0 = _DCV0 + NZ  # 1280
_TOT = _TL0 + I  # 1792

GROUP = 4  # output rows per store DMA (4*128*512*4B = 1 MiB)
NGRP = OL // GROUP
EARLY_GROUPS = 2  # first groups store per-row (256KB) so the write stream starts ASAP
ZSPLIT = NS * GROUP  # Z columns needed by the first store group

_cache: dict = {}


def _build_nc():
    nc = bacc.Bacc("TRN2", target_bir_lowering=False, debug=False, num_devices=N_CORES)
    pk_d = nc.dram_tensor("pk", [128, _TOT], F32, kind="ExternalInput")
    out_d = nc.dram_tensor("out", [OL, B, I], F32, kind="ExternalOutput")

    with tile.TileContext(nc) as tc, ExitStack() as ctx:
        constp = ctx.enter_context(tc.tile_pool(name="const", bufs=1))
        psump = ctx.enter_context(
            tc.tile_pool(name="psum", bufs=1, space=bass.MemorySpace.PSUM)
        )
        outp = ctx.enter_context(tc.tile_pool(name="outs", bufs=1))

        pk = constp.tile([128, _TOT], F32)
        # input loads split across both HWDGE rings (SyncE + ScalarE) so the
        # issue overhead overlaps and the first matmul starts earliest
        # cv first on the ScalarE ring: the GpSimd dcvT sub and therefore the
        # [Z|dZ] matmul are on the critical path to the first output row
        nc.sync.dma_start(pk[:, 0:256], pk_d[:, 0:256])
        nc.scalar.dma_start(pk[:, _CV0 : _CV0 + NZ], pk_d[:, _CV0 : _CV0 + NZ])
        nc.sync.dma_start(pk[:, 512:768], pk_d[:, 512:768])
        nc.scalar.dma_start(pk[:, 256:512], pk_d[:, 256:512])
        nc.scalar.dma_start(pk[:, 768:1024], pk_d[:, 768:1024])
        nc.sync.dma_start(pk[:, _TL0:_TOT], pk_d[:, _TL0:_TOT])

        # dcvT[c, i] = cvT[c, i+1] - cvT[c, i]  (GpSimd, off the critical path)
        nc.gpsimd.memset(pk[:, _DCV0 + NZ - 1 : _DCV0 + NZ], 0.0)  # last dcv col
        nc.gpsimd.tensor_sub(
            pk[:, _DCV0 : _DCV0 + NZ - 1],
            pk[:, _CV0 + 1 : _CV0 + NZ],
            pk[:, _CV0 : _CV0 + NZ - 1],
        )

        # y[c,b] = sum_f E[c,f] x[b,f]: accumulate over 4 chunks of f.
        y_ps = psump.tile([128, B], F32)
        for k in range(4):
            base = k * 256
            nc.tensor.matmul(
                y_ps[:],
                pk[:, base + B : base + B + C],  # lhsT [f_chunk, c]
                pk[:, base : base + B],  # rhs  [f_chunk, b]
                start=(k == 0),
                stop=(k == 3),
            )
        y_sb = constp.tile([128, B], F32)
        nc.vector.tensor_copy(y_sb[:], y_ps[:])

        # One PE pass produces both ZT[b, o*4+s] and dZT[b, o*4+s] by using
        # rhs = [cvT block | dcvT block] (2-block access pattern).  Split
        # 16/112 columns so output row 0 unblocks early.
        cvd = pk[:, _CV0 : _CV0 + 2 * NZ].rearrange("p (u c) -> p u c", u=2)
        ztdz = constp.tile([128, 2 * NZ], F32)  # [ZT | dZT]
        ztdz_v = ztdz[:].rearrange("p (u c) -> p u c", u=2)
        zz_ps1 = psump.tile([128, 2 * ZSPLIT], F32)
        zz_ps2 = psump.tile([128, 2 * (NZ - ZSPLIT)], F32)

        nc.tensor.matmul(
            zz_ps1[:], y_sb[:], cvd[:, :, 0:ZSPLIT], start=True, stop=True
        )
        nc.vector.tensor_copy(
            ztdz_v[:, :, 0:ZSPLIT], zz_ps1[:].rearrange("p (u c) -> p u c", u=2)
        )

        def _ztdz_rest():
            nc.tensor.matmul(
                zz_ps2[:], y_sb[:], cvd[:, :, ZSPLIT:NZ], start=True, stop=True
            )
            nc.scalar.activation(
                ztdz_v[:, :, ZSPLIT:NZ],
                zz_ps2[:].rearrange("p (u c) -> p u c", u=2),
                mybir.ActivationFunctionType.Identity,
            )

        outs = outp.tile([128, OL * I], F32)
        tl_ap = pk[:, _TL0 : _TL0 + I]

        for g in range(NGRP):
            if g == 1:
                _ztdz_rest()
            for oi in range(GROUP):
                o = g * GROUP + oi
                col = o * I
                zc = NS * o
                for (j, t0, t1), eng in zip(_SPANS, _SPAN_ENG):
                    if eng == "a":
                        nc.scalar.activation(
                            outs[:, col + t0 : col + t1],
                            tl_ap[:, t0:t1],
                            mybir.ActivationFunctionType.Identity,
                            bias=ztdz[:, zc + j : zc + j + 1],
                            scale=ztdz[:, NZ + zc + j : NZ + zc + j + 1],
                        )
                    else:
                        veng = nc.vector if eng == "v" else nc.gpsimd
                        veng.tensor_scalar(
                            outs[:, col + t0 : col + t1],
                            tl_ap[:, t0:t1],
                            ztdz[:, NZ + zc + j : NZ + zc + j + 1],
                            ztdz[:, zc + j : zc + j + 1],
                            mybir.AluOpType.mult,
                            mybir.AluOpType.add,
                        )
                if g < EARLY_GROUPS:
                    nc.sync.dma_start(
                        out_d[o : o + 1].rearrange("o b t -> b o t"),
                        outs[:, o * I : (o + 1) * I].rearrange("p (o t) -> p o t", o=1),
                    )
            if g >= EARLY_GROUPS:
                nc.sync.dma_start(
                    out_d[g * GROUP : (g + 1) * GROUP].rearrange("o b t -> b o t"),
                    outs[:, g * GROUP * I : (g + 1) * GROUP * I].rearrange(
                        "p (o t) -> p o t", o=GROUP
                    ),
                )

    nc.compile()
    return nc


def _get_nc():
    if "nc" not in _cache:
        _cache["nc"] = _build_nc()
    return _cache["nc"]


def _pack_inputs(x, control_values, expansion_matrix):
    x = np.ascontiguousarray(x, dtype=np.float32)
    cv = np.ascontiguousarray(control_values, dtype=np.float32)
    E = np.ascontiguousarray(expansion_matrix, dtype=np.float32)

    base = np.zeros((128, _TOT), dtype=np.float32)
    for k in range(4):
        base[:, k * 256 : k * 256 + B] = x[:, k * 128 : (k + 1) * 128].T
        base[:, k * 256 + B : k * 256 + B + C] = E[:, k * 128 : (k + 1) * 128].T
    base[:, _TL0 : _TL0 + I] = _TL[None, :]

    in_maps = []
    for core in range(N_CORES):
        m = base.copy()
        slab = cv[core * OL : (core + 1) * OL].reshape(OL * NS, C)  # [(o,s), c]
        m[:, _CV0 : _CV0 + NZ] = slab.T
        in_maps.append({"pk": m})
    return in_maps


def _run(in_maps, trace=False):
    nc = _get_nc()
    return run_bass_kernel_spmd(
        nc, in_maps, core_ids=list(range(N_CORES)), trace=trace
    )


def kernel(x, control_points, control_values, expansion_matrix):
    in_maps = _pack_inputs(x, control_values, expansion_matrix)
    res = _run(in_maps, trace=False)
    return np.concatenate([r["out"] for r in res.results], axis=0)


def kernel_traced(x, control_points, control_values, expansion_matrix):
    """Same as kernel() but profiles on HW; returns (out, BassKernelResults)."""
    in_maps = _pack_inputs(x, control_values, expansion_matrix)
    res = _run(in_maps, trace=True)
    out = np.concatenate([r["out"] for r in res.results], axis=0)
    return out, res
